# revision 19
# baseline (speedup 1.0000x reference)
"""Causal self-attention (B=4, L=2048, D=1024, H=16, HD=64) on 8 TRN2 cores.

Sharding: 8 shards = 4 batches x 2 head-groups (8 heads each). Each core:
  - QKV projection for its 8 heads (Q^T/K^T in [HD, L] layout, V in [L, HD])
  - causal attention per head, softmax without max-subtraction (logits are
    small by construction), row sums via a ones-column appended to V
  - partial output projection with its 512 rows of out_w
Host sums the two partials per batch and adds out_b.

v2: engine-balanced schedule. The attention inner loop (S matmul -> exp on
ACT -> A@V matmul) is software-pipelined (S emitted two key-blocks ahead)
and projection/output-projection matmul groups are interleaved into the
attention chains as fillers so the tensor engine never idles while the
activation engine runs exp. Normalization tail runs on DVE straight out of
PSUM and its PE broadcast is deferred behind the next chain's S streams.
"""

import os
from collections import deque

import numpy as np
import ml_dtypes

B, L, D, H, HD = 4, 2048, 1024, 16, 64
HPC = 8           # heads per core
NCORES = 8
NKB = L // 128    # key blocks of 128

_STATE = {}


def _build_nc(repeat=1):
    import concourse.bass as bass
    import concourse.mybir as mybir
    import concourse.tile as tile
    from concourse import bacc
    from concourse.masks import make_upper_triangular

    f32 = mybir.dt.float32
    bf16 = mybir.dt.bfloat16
    AF = mybir.ActivationFunctionType
    OP = mybir.AluOpType

    nc = bacc.Bacc(None, target_bir_lowering=False)

    xT = nc.dram_tensor("xT", [D, L], bf16, kind="ExternalInput")
    wqk = nc.dram_tensor("wqk", [D, 2 * HPC * HD], bf16, kind="ExternalInput")
    wv = nc.dram_tensor("wv", [D, HPC * HD], bf16, kind="ExternalInput")
    bqk = nc.dram_tensor("bqk", [128, 8], f32, kind="ExternalInput")
    bvb = nc.dram_tensor("bvb", [128, HPC * HD], f32, kind="ExternalInput")
    w2 = nc.dram_tensor("w2", [HPC * HD, D], bf16, kind="ExternalInput")
    out = nc.dram_tensor("out", [L, D], f32, kind="ExternalOutput")

    KO = D // 128  # contraction blocks for the projections

    with tile.TileContext(nc) as tc:
        with (
            tc.tile_pool(name="const", bufs=1) as cpool,
            tc.tile_pool(name="weights", bufs=1) as wpool,
            tc.tile_pool(name="resident", bufs=1) as rpool,
            tc.tile_pool(name="xc", bufs=4) as xcpool,
            tc.tile_pool(name="ework", bufs=3) as epool,
            tc.tile_pool(name="ywork", bufs=3) as ypool,
            tc.tile_pool(name="rwork", bufs=2) as rwpool,
            tc.tile_pool(name="ps_mm", bufs=2, space="PSUM") as ps_mm,
            tc.tile_pool(name="ps_s", bufs=2, space="PSUM") as ps_s,
            tc.tile_pool(name="ps_u", bufs=1, space="PSUM") as ps_u,
        ):
            ones = cpool.tile([1, 128], bf16)
            nc.vector.memset(ones[:], 1.0)
            mask = cpool.tile([128, 128], bf16)
            make_upper_triangular(nc, mask[:], val=1.0, diag=True)

            wqk_sb = wpool.tile([128, KO, 2 * HPC * HD], bf16)
            wv_sb = wpool.tile([128, KO, HPC * HD], bf16)
            bqk_sb = wpool.tile([128, 8], f32)
            bvb_sb = wpool.tile([128, HPC * HD], f32)
            w2_sb = wpool.tile([128, 4, D], bf16)

            # Q^T / K^T packed as head pairs: head h lives at partitions
            # (h%2)*64..+64 of block h//2.
            QT = rpool.tile([128, 4, L], bf16)
            KT = rpool.tile([128, 4, L], bf16)
            # V with a ones column at index 64 (col 65 is alignment padding).
            V = rpool.tile([128, NKB, HPC, 66], bf16)
            nc.vector.memset(V[:, :, :, 64:66], 0.0)
            nc.vector.memset(V[:, :, :, 64:65], 1.0)
            OT = rpool.tile([128, 4, L], bf16)

            xTr = xT.rearrange("(ko p) n -> p ko n", p=128)

            def emit_weight_dmas(jt0_xc):
                # x tile for jt=0 and the first wqk chunk go first so the
                # tensor engine can start; later chunks stream in behind
                # (subtile deps let each ko-pair unblock as it lands).
                # x on the SP queue, weights on the DVE queue: the two DMA
                # rings run in parallel so the first matmul group unblocks at
                # max(x0, wqk chunk0) instead of their sum
                wqk_r = wqk.rearrange("(ko p) m -> p ko m", p=128)
                nc.sync.dma_start(jt0_xc[:, 0:2, :], xTr[:, 0:2, 0:512])
                nc.scalar.dma_start(wqk_sb[:, 0:2, :], wqk_r[:, 0:2, :])
                for c in range(1, 4):
                    nc.sync.dma_start(
                        jt0_xc[:, 2 * c:2 * c + 2, :], xTr[:, 2 * c:2 * c + 2, 0:512])
                nc.scalar.dma_start(bqk_sb[:], bqk[:])
                for c in range(1, 4):
                    nc.scalar.dma_start(
                        wqk_sb[:, 2 * c:2 * c + 2, :], wqk_r[:, 2 * c:2 * c + 2, :])
                wv_r = wv.rearrange("(ko p) m -> p ko m", p=128)
                for c in range(2):
                    nc.scalar.dma_start(
                        wv_sb[:, 4 * c:4 * c + 4, :], wv_r[:, 4 * c:4 * c + 4, :])
                nc.scalar.dma_start(bvb_sb[:], bvb[:])
                nc.scalar.dma_start(w2_sb[:], w2.rearrange("(o p) n -> p o n", p=128))

            def emit_pass(first):
                xc_tiles = {}
                emitted = set()
                groups = {}
                fillers = deque()
                pending = [None]

                def flush():
                    if pending[0] is not None:
                        fn, pending[0] = pending[0], None
                        fn()

                def force(key):
                    if key not in emitted:
                        emitted.add(key)
                        groups[key]()

                pace = [1, 0]  # stride, iteration counter

                def maybe_filler():
                    pace[1] += 1
                    if pace[1] % pace[0]:
                        return
                    while fillers:
                        key = fillers.popleft()
                        if key in emitted:
                            continue
                        emitted.add(key)
                        groups[key]()
                        return

                def prefetch_xc(jt):
                    if jt in xc_tiles or jt > 3:
                        return
                    xc = xcpool.tile([128, KO, 512], bf16)
                    nc.sync.dma_start(xc[:], xTr[:, :, jt * 512:(jt + 1) * 512])
                    xc_tiles[jt] = xc

                def mk_qk(jt, mb):
                    def _g():
                        sl = slice(jt * 512, (jt + 1) * 512)
                        xc = xc_tiles[jt]
                        t = ps_mm.tile([128, 512], f32, tag="mm512")
                        for ko in range(KO):
                            nc.tensor.matmul(
                                t[:],
                                wqk_sb[:, ko, mb * 128:(mb + 1) * 128],
                                xc[:, ko, :],
                                start=(ko == 0),
                                stop=(ko == KO - 1),
                            )
                        dst = QT[:, mb, sl] if mb < 4 else KT[:, mb - 4, sl]
                        nc.vector.tensor_scalar_add(dst, t[:], bqk_sb[:, mb:mb + 1])
                    return _g

                def mk_v(jt, qb):
                    def _g():
                        g = jt * 4 + qb
                        xc = xc_tiles[jt]
                        tv = ps_mm.tile([128, 512], f32, tag="mm512")
                        for ko in range(KO):
                            nc.tensor.matmul(
                                tv[:],
                                xc[:, ko, qb * 128:(qb + 1) * 128],
                                wv_sb[:, ko, :],
                                start=(ko == 0),
                                stop=(ko == KO - 1),
                            )
                        nc.vector.tensor_tensor(
                            out=V[:, g, :, 0:64],
                            in0=tv.rearrange("p (h e) -> p h e", e=HD),
                            in1=bvb_sb[:].rearrange("p (h e) -> p h e", e=HD),
                            op=OP.add,
                        )
                    return _g

                def mk_c(jt, qb, nb):
                    def _g():
                        qa = jt * 4 + qb
                        y_ps = ps_mm.tile([128, 512], f32, tag="mm512")
                        for hp in range(4):
                            nc.tensor.matmul(
                                y_ps[:],
                                OT[:, hp, qa * 128:(qa + 1) * 128],
                                w2_sb[:, hp, nb * 512:(nb + 1) * 512],
                                start=(hp == 0),
                                stop=(hp == 3),
                            )
                        y_sb = ypool.tile([128, 512], f32)
                        if jt == 3 and qb % 2 == 1:
                            # epilogue: ACT is idle, split the evacuations
                            nc.scalar.copy(y_sb[:], y_ps[:])
                        else:
                            nc.vector.tensor_copy(y_sb[:], y_ps[:])
                        dma_eng = nc.scalar if (jt == 3 and nb == 1) else nc.sync
                        dma_eng.dma_start(
                            out[qa * 128:(qa + 1) * 128, nb * 512:(nb + 1) * 512],
                            y_sb[:],
                        )
                    return _g

                for jt in range(4):
                    for mb in range(8):
                        groups[("qk", jt, mb)] = mk_qk(jt, mb)
                    for qb in range(4):
                        groups[("v", jt, qb)] = mk_v(jt, qb)
                    for qb in range(4):
                        for nb in range(2):
                            groups[("c", jt, qb, nb)] = mk_c(jt, qb, nb)

                def chain(hp, jt):
                    nkb = 4 * (jt + 1)
                    force(("qk", jt, hp))
                    force(("qk", jt, 4 + hp))
                    s_tiles = {}

                    def emit_S(kb):
                        q_off = max(0, kb * 128 - jt * 512)
                        qsl = slice(jt * 512 + q_off, (jt + 1) * 512)
                        ksl = slice(kb * 128, (kb + 1) * 128)
                        s = ps_s.tile([128, 2, 512], f32, tag="s_pair")
                        nc.tensor.matmul(
                            s[:, 0, q_off:], KT[0:64, hp, ksl], QT[0:64, hp, qsl],
                            start=True, stop=True,
                        )
                        nc.tensor.matmul(
                            s[:, 1, q_off:], KT[64:128, hp, ksl], QT[64:128, hp, qsl],
                            start=True, stop=True,
                        )
                        s_tiles[kb] = (s, q_off)

                    emit_S(0)
                    if nkb > 1:
                        emit_S(1)
                    flush()
                    u_e = ps_u.tile([65, 512], f32, tag="u_e")
                    u_o = ps_u.tile([65, 512], f32, tag="u_o")
                    for kb in range(nkb):
                        maybe_filler()
                        if kb + 2 < nkb:
                            emit_S(kb + 2)
                        s, q_off = s_tiles.pop(kb)
                        et = epool.tile([128, 2, 512], bf16, tag="et")
                        nc.scalar.activation(
                            et[:, :, q_off:], s[:, :, q_off:], AF.Exp, scale=0.125)
                        if kb * 128 >= jt * 512:  # diagonal block
                            force(("v", jt, kb - jt * 4))
                            nc.vector.tensor_tensor(
                                out=et[:, :, q_off:q_off + 128],
                                in0=et[:, :, q_off:q_off + 128],
                                in1=mask[:, None, :].broadcast_to([128, 2, 128]),
                                op=OP.mult,
                            )
                        nc.tensor.matmul(
                            u_e[:, q_off:], V[:, kb, 2 * hp, 0:65], et[:, 0, q_off:],
                            start=(kb == 0), stop=(kb == nkb - 1),
                        )
                        nc.tensor.matmul(
                            u_o[:, q_off:], V[:, kb, 2 * hp + 1, 0:65],
                            et[:, 1, q_off:],
                            start=(kb == 0), stop=(kb == nkb - 1),
                        )

                    def norm_tail():
                        sl = slice(jt * 512, (jt + 1) * 512)
                        # evacuate U fast on ACT (idle at chain boundaries) so
                        # the next chain's first U matmul isn't WAR-blocked on
                        # the whole normalization
                        u_sbs = []
                        for side, u_ps in ((0, u_e), (1, u_o)):
                            u_sb = rwpool.tile([65, 512], f32, tag=f"usb{side}")
                            nc.scalar.copy(u_sb[:], u_ps[:])
                            u_sbs.append(u_sb)
                        for side, u_sb in ((0, u_sbs[0]), (1, u_sbs[1])):
                            rcp = rwpool.tile([1, 512], f32, tag="rcp")
                            nc.vector.reciprocal(rcp[:], u_sb[64:65, :])
                            rb = rwpool.tile([64, 512], f32, tag="rb")
                            nc.gpsimd.partition_broadcast(rb[:], rcp[0:1, :])
                            nc.vector.tensor_tensor(
                                out=OT[side * 64:side * 64 + 64, hp, sl],
                                in0=u_sb[0:64, :],
                                in1=rb[:],
                                op=OP.mult,
                            )
                    pending[0] = norm_tail

                # ---- schedule ----
                if first:
                    xc0 = xcpool.tile([128, KO, 512], bf16)
                    xc_tiles[0] = xc0
                    emit_weight_dmas(xc0)
                else:
                    prefetch_xc(0)
                prefetch_xc(1)
                force(("qk", 0, 0))
                force(("qk", 0, 4))
                force(("v", 0, 0))
                fillers.extend(
                    [("v", 0, qb) for qb in (1, 2, 3)]
                    + [("qk", 0, mb) for mb in (1, 5, 2, 6, 3, 7)])
                for jt in range(4):
                    if jt >= 1:
                        prefetch_xc(jt + 1)
                    # all output projections go to the last round: its chains
                    # have the most kb iterations but no projection work left
                    if jt == 3:
                        fillers.extend(
                            [("c", cj, qb, nb) for cj in range(3)
                             for qb in range(4) for nb in range(2)])
                    if jt + 1 <= 3:
                        fillers.extend(
                            [("qk", jt + 1, mb) for mb in (0, 4, 1, 5, 2, 6, 3, 7)]
                            + [("v", jt + 1, qb) for qb in range(4)])
                    iters = 4 * 4 * (jt + 1)
                    navail = sum(1 for k in fillers if k not in emitted)
                    pace[0] = max(1, iters // max(1, navail))
                    pace[1] = 0
                    for hp in range(4):
                        chain(hp, jt)
                flush()
                while fillers:
                    maybe_filler()
                for qb in range(4):
                    for nb in range(2):
                        force(("c", 3, qb, nb))

            for _rep in range(repeat):
                emit_pass(_rep == 0)
    nc.compile()
    return nc


def _get_nc():
    if "nc" not in _STATE:
        _STATE["nc"] = _build_nc()
    return _STATE["nc"]


def make_in_maps(x, in_w, in_b, out_w, out_b):
    bf = ml_dtypes.bfloat16
    x = np.asarray(x, dtype=np.float32)
    in_w = np.asarray(in_w, dtype=np.float32)
    in_b = np.asarray(in_b, dtype=np.float32)
    out_w = np.asarray(out_w, dtype=np.float32)

    in_maps = []
    for c in range(NCORES):
        b, hg = c // 2, c % 2
        hsl = slice(hg * HPC * HD, (hg + 1) * HPC * HD)  # 512 cols of each section
        wq = in_w[:, 0:D][:, hsl]
        wk = in_w[:, D:2 * D][:, hsl]
        wv_ = in_w[:, 2 * D:3 * D][:, hsl]
        bq = in_b[0:D][hsl]
        bk = in_b[D:2 * D][hsl]
        bv_ = in_b[2 * D:3 * D][hsl]
        in_maps.append({
            "xT": np.ascontiguousarray(x[b].T).astype(bf),
            "wqk": np.ascontiguousarray(
                np.concatenate([wq, wk], axis=1)).astype(bf),
            "wv": np.ascontiguousarray(wv_).astype(bf),
            "bqk": np.ascontiguousarray(
                np.concatenate([bq, bk]).reshape(8, 128).T).astype(np.float32),
            "bvb": np.ascontiguousarray(
                np.broadcast_to(bv_[None, :], (128, HPC * HD))).astype(np.float32),
            "w2": np.ascontiguousarray(out_w[hsl, :]).astype(bf),
        })
    return in_maps


def kernel(x, in_w, in_b, out_w, out_b):
    from concourse.bass_utils import run_bass_kernel_spmd

    out_b = np.asarray(out_b, dtype=np.float32)
    nc = _get_nc()
    in_maps = make_in_maps(x, in_w, in_b, out_w, out_b)

    trace = bool(int(os.environ.get("KERNEL_TRACE", "0")))
    if not trace:
        # the axon NTFF profile hook is absent in this container; make sure a
        # stray BASS_TRACE=1 in the environment can't route us into it
        os.environ["BASS_NEVER_TRACE"] = "1"
    res = run_bass_kernel_spmd(
        nc, in_maps, core_ids=list(range(NCORES)), trace=trace,
    )
    _STATE["last_result"] = res
    _STATE["last_in_maps"] = in_maps

    y = np.zeros((B, L, D), dtype=np.float32)
    for c in range(NCORES):
        y[c // 2] += res.results[c]["out"]
    y += out_b[None, None, :]
    return y


# revision 20
# speedup vs baseline: 1.2456x; 1.2456x over previous
"""Causal self-attention (B=4, L=2048, D=1024, H=16, HD=64) on 8 TRN2 cores.

Sharding: 8 shards = 4 batches x 2 head-groups (8 heads each). Each core:
  - QKV projection for its 8 heads (Q^T/K^T in [HD, L] layout, V in [L, HD])
  - causal attention per head, softmax without max-subtraction (logits are
    small by construction), row sums via a ones-column appended to V
  - partial output projection with its 512 rows of out_w
Host sums the two partials per batch and adds out_b.

v2: engine-balanced schedule. The attention inner loop (S matmul -> exp on
ACT -> A@V matmul) is software-pipelined (S emitted two key-blocks ahead)
and projection/output-projection matmul groups are interleaved into the
attention chains as fillers so the tensor engine never idles while the
activation engine runs exp. Normalization tail runs on DVE straight out of
PSUM and its PE broadcast is deferred behind the next chain's S streams.
"""

import os
from collections import deque

import numpy as np
import ml_dtypes

B, L, D, H, HD = 4, 2048, 1024, 16, 64
HPC = 8           # heads per core
NCORES = 8
NKB = L // 128    # key blocks of 128

_STATE = {}


def _build_nc(repeat=1):
    import concourse.bass as bass
    import concourse.mybir as mybir
    import concourse.tile as tile
    from concourse import bacc
    from concourse.masks import make_upper_triangular

    f32 = mybir.dt.float32
    bf16 = mybir.dt.bfloat16
    AF = mybir.ActivationFunctionType
    OP = mybir.AluOpType

    nc = bacc.Bacc(None, target_bir_lowering=False)

    xT = nc.dram_tensor("xT", [D, L], bf16, kind="ExternalInput")
    wqk = nc.dram_tensor("wqk", [D, 2 * HPC * HD], bf16, kind="ExternalInput")
    wv = nc.dram_tensor("wv", [D, HPC * HD], bf16, kind="ExternalInput")
    bqk = nc.dram_tensor("bqk", [128, 8], f32, kind="ExternalInput")
    bvb = nc.dram_tensor("bvb", [128, HPC * HD], f32, kind="ExternalInput")
    w2 = nc.dram_tensor("w2", [HPC * HD, D], bf16, kind="ExternalInput")
    out = nc.dram_tensor("out", [L, D], f32, kind="ExternalOutput")

    KO = D // 128  # contraction blocks for the projections

    with tile.TileContext(nc) as tc:
        with (
            tc.tile_pool(name="const", bufs=1) as cpool,
            tc.tile_pool(name="weights", bufs=1) as wpool,
            tc.tile_pool(name="resident", bufs=1) as rpool,
            tc.tile_pool(name="xc", bufs=4) as xcpool,
            tc.tile_pool(name="ework", bufs=3) as epool,
            tc.tile_pool(name="ywork", bufs=3) as ypool,
            tc.tile_pool(name="rwork", bufs=2) as rwpool,
            tc.tile_pool(name="ps_mm", bufs=2, space="PSUM") as ps_mm,
            tc.tile_pool(name="ps_s", bufs=2, space="PSUM") as ps_s,
            tc.tile_pool(name="ps_u", bufs=1, space="PSUM") as ps_u,
        ):
            ones = cpool.tile([1, 128], bf16)
            nc.vector.memset(ones[:], 1.0)
            mask = cpool.tile([128, 128], bf16)
            make_upper_triangular(nc, mask[:], val=1.0, diag=True)

            wqk_sb = wpool.tile([128, KO, 2 * HPC * HD], bf16)
            wv_sb = wpool.tile([128, KO, HPC * HD], bf16)
            bqk_sb = wpool.tile([128, 8], f32)
            bvb_sb = wpool.tile([128, HPC * HD], f32)
            w2_sb = wpool.tile([128, 4, D], bf16)

            # Q^T / K^T packed as head pairs: head h lives at partitions
            # (h%2)*64..+64 of block h//2.
            QT = rpool.tile([128, 4, L], bf16)
            KT = rpool.tile([128, 4, L], bf16)
            # V with a ones column at index 64 (col 65 is alignment padding).
            V = rpool.tile([128, NKB, HPC, 66], bf16)
            nc.vector.memset(V[:, :, :, 64:66], 0.0)
            nc.vector.memset(V[:, :, :, 64:65], 1.0)
            OT = rpool.tile([128, 4, L], bf16)

            xTr = xT.rearrange("(ko p) n -> p ko n", p=128)

            def emit_weight_dmas(jt0_xc):
                # x tile for jt=0 and the first wqk chunk go first so the
                # tensor engine can start; later chunks stream in behind
                # (subtile deps let each ko-pair unblock as it lands).
                # x on the SP queue, weights on the DVE queue: the two DMA
                # rings run in parallel so the first matmul group unblocks at
                # max(x0, wqk chunk0) instead of their sum
                wqk_r = wqk.rearrange("(ko p) m -> p ko m", p=128)
                nc.sync.dma_start(jt0_xc[:, 0:2, :], xTr[:, 0:2, 0:512])
                nc.scalar.dma_start(wqk_sb[:, 0:2, :], wqk_r[:, 0:2, :])
                for c in range(1, 4):
                    nc.sync.dma_start(
                        jt0_xc[:, 2 * c:2 * c + 2, :], xTr[:, 2 * c:2 * c + 2, 0:512])
                nc.scalar.dma_start(bqk_sb[:], bqk[:])
                for c in range(1, 4):
                    nc.scalar.dma_start(
                        wqk_sb[:, 2 * c:2 * c + 2, :], wqk_r[:, 2 * c:2 * c + 2, :])
                wv_r = wv.rearrange("(ko p) m -> p ko m", p=128)
                for c in range(2):
                    nc.scalar.dma_start(
                        wv_sb[:, 4 * c:4 * c + 4, :], wv_r[:, 4 * c:4 * c + 4, :])
                nc.scalar.dma_start(bvb_sb[:], bvb[:])
                nc.scalar.dma_start(w2_sb[:], w2.rearrange("(o p) n -> p o n", p=128))

            def emit_pass(first):
                xc_tiles = {}
                emitted = set()
                groups = {}
                fillers = deque()
                pending = [None]

                def flush():
                    if pending[0] is not None:
                        fn, pending[0] = pending[0], None
                        fn()

                def force(key):
                    if key not in emitted:
                        emitted.add(key)
                        groups[key]()

                pace = [1, 0]  # stride, iteration counter

                def maybe_filler():
                    pace[1] += 1
                    if pace[1] % pace[0]:
                        return
                    while fillers:
                        key = fillers.popleft()
                        if key in emitted:
                            continue
                        emitted.add(key)
                        groups[key]()
                        return

                def prefetch_xc(jt):
                    if jt in xc_tiles or jt > 3:
                        return
                    xc = xcpool.tile([128, KO, 512], bf16)
                    nc.sync.dma_start(xc[:], xTr[:, :, jt * 512:(jt + 1) * 512])
                    xc_tiles[jt] = xc

                def mk_qk(jt, mb):
                    def _g():
                        sl = slice(jt * 512, (jt + 1) * 512)
                        xc = xc_tiles[jt]
                        t = ps_mm.tile([128, 512], f32, tag="mm512")
                        for ko in range(KO):
                            nc.tensor.matmul(
                                t[:],
                                wqk_sb[:, ko, mb * 128:(mb + 1) * 128],
                                xc[:, ko, :],
                                start=(ko == 0),
                                stop=(ko == KO - 1),
                            )
                        dst = QT[:, mb, sl] if mb < 4 else KT[:, mb - 4, sl]
                        nc.vector.tensor_scalar_add(dst, t[:], bqk_sb[:, mb:mb + 1])
                    return _g

                def mk_v(jt, qb):
                    def _g():
                        g = jt * 4 + qb
                        xc = xc_tiles[jt]
                        tv = ps_mm.tile([128, 512], f32, tag="mm512")
                        for ko in range(KO):
                            nc.tensor.matmul(
                                tv[:],
                                xc[:, ko, qb * 128:(qb + 1) * 128],
                                wv_sb[:, ko, :],
                                start=(ko == 0),
                                stop=(ko == KO - 1),
                            )
                        nc.vector.tensor_tensor(
                            out=V[:, g, :, 0:64],
                            in0=tv.rearrange("p (h e) -> p h e", e=HD),
                            in1=bvb_sb[:].rearrange("p (h e) -> p h e", e=HD),
                            op=OP.add,
                        )
                    return _g

                def mk_c(jt, qb, nb):
                    def _g():
                        qa = jt * 4 + qb
                        y_ps = ps_mm.tile([128, 512], f32, tag="mm512")
                        for hp in range(4):
                            nc.tensor.matmul(
                                y_ps[:],
                                OT[:, hp, qa * 128:(qa + 1) * 128],
                                w2_sb[:, hp, nb * 512:(nb + 1) * 512],
                                start=(hp == 0),
                                stop=(hp == 3),
                            )
                        y_sb = ypool.tile([128, 512], f32)
                        if jt == 3 and qb % 2 == 1:
                            # epilogue: ACT is idle, split the evacuations
                            nc.scalar.copy(y_sb[:], y_ps[:])
                        else:
                            nc.vector.tensor_copy(y_sb[:], y_ps[:])
                        nc.sync.dma_start(
                            out[qa * 128:(qa + 1) * 128, nb * 512:(nb + 1) * 512],
                            y_sb[:],
                        )
                    return _g

                for jt in range(4):
                    for mb in range(8):
                        groups[("qk", jt, mb)] = mk_qk(jt, mb)
                    for qb in range(4):
                        groups[("v", jt, qb)] = mk_v(jt, qb)
                    for qb in range(4):
                        for nb in range(2):
                            groups[("c", jt, qb, nb)] = mk_c(jt, qb, nb)

                def chain(hp, jt):
                    nkb = 4 * (jt + 1)
                    force(("qk", jt, hp))
                    force(("qk", jt, 4 + hp))
                    s_tiles = {}

                    def emit_S(kb):
                        q_off = max(0, kb * 128 - jt * 512)
                        qsl = slice(jt * 512 + q_off, (jt + 1) * 512)
                        ksl = slice(kb * 128, (kb + 1) * 128)
                        s = ps_s.tile([128, 2, 512], f32, tag="s_pair")
                        nc.tensor.matmul(
                            s[:, 0, q_off:], KT[0:64, hp, ksl], QT[0:64, hp, qsl],
                            start=True, stop=True,
                        )
                        nc.tensor.matmul(
                            s[:, 1, q_off:], KT[64:128, hp, ksl], QT[64:128, hp, qsl],
                            start=True, stop=True,
                        )
                        s_tiles[kb] = (s, q_off)

                    emit_S(0)
                    if nkb > 1:
                        emit_S(1)
                    flush()
                    u_e = ps_u.tile([65, 512], f32, tag="u_e")
                    u_o = ps_u.tile([65, 512], f32, tag="u_o")
                    for kb in range(nkb):
                        maybe_filler()
                        if kb + 2 < nkb:
                            emit_S(kb + 2)
                        s, q_off = s_tiles.pop(kb)
                        et = epool.tile([128, 2, 512], bf16, tag="et")
                        nc.scalar.activation(
                            et[:, :, q_off:], s[:, :, q_off:], AF.Exp, scale=0.125)
                        if kb * 128 >= jt * 512:  # diagonal block
                            force(("v", jt, kb - jt * 4))
                            nc.vector.tensor_tensor(
                                out=et[:, :, q_off:q_off + 128],
                                in0=et[:, :, q_off:q_off + 128],
                                in1=mask[:, None, :].broadcast_to([128, 2, 128]),
                                op=OP.mult,
                            )
                        nc.tensor.matmul(
                            u_e[:, q_off:], V[:, kb, 2 * hp, 0:65], et[:, 0, q_off:],
                            start=(kb == 0), stop=(kb == nkb - 1),
                        )
                        nc.tensor.matmul(
                            u_o[:, q_off:], V[:, kb, 2 * hp + 1, 0:65],
                            et[:, 1, q_off:],
                            start=(kb == 0), stop=(kb == nkb - 1),
                        )

                    def norm_tail():
                        sl = slice(jt * 512, (jt + 1) * 512)
                        # evacuate U fast on ACT (idle at chain boundaries) so
                        # the next chain's first U matmul isn't WAR-blocked on
                        # the whole normalization
                        u_sbs = []
                        for side, u_ps in ((0, u_e), (1, u_o)):
                            u_sb = rwpool.tile([65, 512], f32, tag=f"usb{side}")
                            nc.scalar.copy(u_sb[:], u_ps[:])
                            u_sbs.append(u_sb)
                        for side, u_sb in ((0, u_sbs[0]), (1, u_sbs[1])):
                            rcp = rwpool.tile([1, 512], f32, tag="rcp")
                            nc.vector.reciprocal(rcp[:], u_sb[64:65, :])
                            rb = rwpool.tile([64, 512], f32, tag="rb")
                            nc.gpsimd.partition_broadcast(rb[:], rcp[0:1, :])
                            nc.vector.tensor_tensor(
                                out=OT[side * 64:side * 64 + 64, hp, sl],
                                in0=u_sb[0:64, :],
                                in1=rb[:],
                                op=OP.mult,
                            )
                    pending[0] = norm_tail

                # ---- schedule ----
                if first:
                    xc0 = xcpool.tile([128, KO, 512], bf16)
                    xc_tiles[0] = xc0
                    emit_weight_dmas(xc0)
                else:
                    prefetch_xc(0)
                prefetch_xc(1)
                force(("qk", 0, 0))
                force(("qk", 0, 4))
                force(("v", 0, 0))
                fillers.extend(
                    [("v", 0, qb) for qb in (1, 2, 3)]
                    + [("qk", 0, mb) for mb in (1, 5, 2, 6, 3, 7)])
                for jt in range(4):
                    if jt >= 1:
                        prefetch_xc(jt + 1)
                    # all output projections go to the last round: its chains
                    # have the most kb iterations but no projection work left
                    if jt == 3:
                        fillers.extend(
                            [("c", cj, qb, nb) for cj in range(3)
                             for qb in range(4) for nb in range(2)])
                    if jt + 1 <= 3:
                        fillers.extend(
                            [("qk", jt + 1, mb) for mb in (0, 4, 1, 5, 2, 6, 3, 7)]
                            + [("v", jt + 1, qb) for qb in range(4)])
                    iters = 4 * 4 * (jt + 1)
                    navail = sum(1 for k in fillers if k not in emitted)
                    pace[0] = max(1, iters // max(1, navail))
                    pace[1] = 0
                    for hp in range(4):
                        chain(hp, jt)
                flush()
                while fillers:
                    maybe_filler()
                for qb in range(4):
                    for nb in range(2):
                        force(("c", 3, qb, nb))

            for _rep in range(repeat):
                emit_pass(_rep == 0)
    nc.compile()
    return nc


def _get_nc():
    if "nc" not in _STATE:
        _STATE["nc"] = _build_nc()
    return _STATE["nc"]


def make_in_maps(x, in_w, in_b, out_w, out_b):
    bf = ml_dtypes.bfloat16
    x = np.asarray(x, dtype=np.float32)
    in_w = np.asarray(in_w, dtype=np.float32)
    in_b = np.asarray(in_b, dtype=np.float32)
    out_w = np.asarray(out_w, dtype=np.float32)

    in_maps = []
    for c in range(NCORES):
        b, hg = c // 2, c % 2
        hsl = slice(hg * HPC * HD, (hg + 1) * HPC * HD)  # 512 cols of each section
        wq = in_w[:, 0:D][:, hsl]
        wk = in_w[:, D:2 * D][:, hsl]
        wv_ = in_w[:, 2 * D:3 * D][:, hsl]
        bq = in_b[0:D][hsl]
        bk = in_b[D:2 * D][hsl]
        bv_ = in_b[2 * D:3 * D][hsl]
        in_maps.append({
            "xT": np.ascontiguousarray(x[b].T).astype(bf),
            "wqk": np.ascontiguousarray(
                np.concatenate([wq, wk], axis=1)).astype(bf),
            "wv": np.ascontiguousarray(wv_).astype(bf),
            "bqk": np.ascontiguousarray(
                np.concatenate([bq, bk]).reshape(8, 128).T).astype(np.float32),
            "bvb": np.ascontiguousarray(
                np.broadcast_to(bv_[None, :], (128, HPC * HD))).astype(np.float32),
            "w2": np.ascontiguousarray(out_w[hsl, :]).astype(bf),
        })
    return in_maps


def kernel(x, in_w, in_b, out_w, out_b):
    from concourse.bass_utils import run_bass_kernel_spmd

    out_b = np.asarray(out_b, dtype=np.float32)
    nc = _get_nc()
    in_maps = make_in_maps(x, in_w, in_b, out_w, out_b)

    trace = bool(int(os.environ.get("KERNEL_TRACE", "0")))
    if not trace:
        # the axon NTFF profile hook is absent in this container; make sure a
        # stray BASS_TRACE=1 in the environment can't route us into it
        os.environ["BASS_NEVER_TRACE"] = "1"
    res = run_bass_kernel_spmd(
        nc, in_maps, core_ids=list(range(NCORES)), trace=trace,
    )
    _STATE["last_result"] = res
    _STATE["last_in_maps"] = in_maps

    y = np.zeros((B, L, D), dtype=np.float32)
    for c in range(NCORES):
        y[c // 2] += res.results[c]["out"]
    y += out_b[None, None, :]
    return y
